# revision 19
# baseline (speedup 1.0000x reference)
"""BlurDownsample (depthwise 4x4 FIR + 2x downsample) on 8 TRN2 NeuronCores.

Contract: kernel(x, f) takes the FULL inputs
    x: [16, 128, 256, 256] float32,  f: [4, 4] float32
and returns the FULL output [16, 128, 128, 128] float32, matching
    upfirdn2d(x, f, down=2, padding=(1, 1), flip_filter=False):
    out[n,c,oy,ox] = sum_{dy,dx in 0..3} f[3-dy, 3-dx] * x[2oy+dy-1, 2ox+dx-1]
(out-of-range x indices read as zero).

Sharding: pure data-parallel over the batch - core k processes
x[2k:2k+2]; outputs are reassembled on the host.

Fast path (separable filter, used for the graded f = outer([1,3,3,1])^2):
  f is SVD-decomposed on the HOST into rank-1 f = fv ⊗ fh. The device
  program then does:
  - Vertical FIR on the Tensor engine: banded bf16 matrices
        B[r,ph][p, oh] = fv[3-dy] * s_ph,   dy = 2p + r - 2oh + 1
    (s_ph folds the horizontal center taps fh[2]/fh[1] in) contract the
    input-row pairs held on partitions. rhs is the even/odd (ph) column
    phase of x, so PSUM receives two planes per channel:
        e'[oh,k] = fh[2] * vert(x)[oh, 2k],  o'[oh,k] = fh[1] * vert[2k+1]
    Only 4 matmuls x 256 columns per channel pair (vs 8 in the direct
    2D formulation) - each input column is streamed exactly once, which
    halves Tensor-engine columns/power. That matters because the
    baseline trace showed HAM power throttling (k=4/8 duty for ~57us)
    gating the Tensor engine and stalling the whole load pipeline.
  - PSUM layout is PH-MAJOR [oh, ph, c, k] so each 4-channel matmul fills
    exactly ONE 2KB PSUM bank (a start=True reset is bank-scoped; mixed-
    region banks with multi-bank matmuls corrupt accumulation).
  - Horizontal 4-tap combine, engine-balanced (measured: DVE STT has no
    fast mode ~1.2ns/elem, DVE TT gets 2x for bf16 SBUF, Pool only does
    plain TT at 0.42 efficiency, only DVE/Act can read PSUM):
      Scalar engine: ONE activation copy drains the whole PSUM tile to
        bf16 SBUF (psum frees immediately -> short cross-engine chain);
      Pool (gpsimd): tt = e' + o' (plain add);
      DVE: tt[1:] += (fh[3]/fh[1]) o'[:-1],
           acc[:-1] = (fh[0]/fh[2]) e'[1:] + tt[:-1], 1-col fixup.
    This caps every engine at ~83us/core: the chip HAM power throttle
    (k=4/8 duty windows, 40-60% of runtime on a busy box) gates compute
    engines but NOT the DMA queues, so exec ~ head + maxEngine/(1-thr)
    + tail; balance matters more than total work.
  - B matrices are computed on the host and passed as a bf16 input:
    ZERO on-device filter setup, so the first matmul only waits for the
    first (deliberately small) x group - the baseline lost 54us waiting
    for B cast-DMAs queued behind 8 MiB loads.
  - x is cast fp32 -> bf16 inside the load DMA (SWDGE on the gpsimd
    queue) with two adjacent H rows per partition => 2 KB contiguous
    HBM bursts (~410 GB/s observed). Group sizes taper 8,8,16,32,... at
    the start (compute starts ~11us in) and ...,16,8,8 at the end
    (short drain tail).
  - Output is stored bf16 in [n, oh, c, ow] layout (4-8 KB contiguous
    runs on the sync HWDGE queue); the host transposes back and
    upcasts (rel err ~4e-3 vs the 2e-2 gate).

Fallback path (non-separable f): the previous direct-2D kernel (whole
4x4 FIR as 8 banded matmuls per channel pair, B built on device).

Roofline: 72 MiB HBM traffic (64 read + 8 write) at the ~410-435 GB/s
per-core DMA fabric rate = 174-184us, plus ~10us head/tail.
"""

from contextlib import ExitStack

import numpy as np
import ml_dtypes

import concourse.tile as tile
from concourse import bacc, mybir
from concourse.bass_utils import run_bass_kernel_spmd

F32 = mybir.dt.float32
BF16 = mybir.dt.bfloat16

N_CORES = 8


def _group_sizes(nb, C):
    """Per-image channel-group sizes: taper up at the very start (so the
    first matmul starts early) and down at the very end (short tail)."""
    assert C == 128, "tuned for C=128"
    head = [8, 8, 16, 32, 32, 32]
    full = [32, 32, 32, 32]
    tail = [32, 32, 32, 16, 8, 8]
    both = [8, 8, 16, 32, 32, 16, 8, 8]
    if nb == 1:
        imgs = [both]
    else:
        imgs = [head] + [full] * (nb - 2) + [tail]
    assert all(sum(g) == C for g in imgs)
    return imgs


# ---------------------------------------------------------------------------
# Fast path: separable filter, host-built B matrices
# ---------------------------------------------------------------------------


def _host_separable(f):
    """Rank-1 decomposition of f with folded horizontal scales.

    Returns (B_host [128, 4, 128] bf16, r0, r3) or None if f is not
    (numerically) rank-1 / has degenerate center taps.
    """
    f64 = np.asarray(f, dtype=np.float64)
    U, S, Vt = np.linalg.svd(f64)
    if S[0] == 0.0 or S[1] > 1e-5 * S[0]:
        return None
    fv = U[:, 0] * S[0]
    fh = Vt[0, :]
    if fh.sum() < 0:
        fv, fh = -fv, -fh
    m = np.abs(fh).max()
    se, so = fh[2], fh[1]
    if abs(se) < 1e-3 * m or abs(so) < 1e-3 * m:
        return None
    r0 = fh[0] / se
    r3 = fh[3] / so
    # B[r,ph][p,oh] = fv[3-dy] * s_ph, dy = 2p + r - 2oh + 1 (0<=dy<=3)
    p = np.arange(128)[:, None]
    oh = np.arange(128)[None, :]
    B = np.zeros((2, 2, 128, 128), np.float64)
    for r in range(2):
        dy = 2 * p + r - 2 * oh + 1
        valid = (dy >= 0) & (dy <= 3)
        vals = np.where(valid, fv[np.clip(3 - dy, 0, 3)], 0.0)
        B[r, 0] = vals * se
        B[r, 1] = vals * so
    # pack to [p, b=2r+ph, oh]
    Bp = np.ascontiguousarray(np.transpose(B, (2, 0, 1, 3)).reshape(128, 4, 128))
    return Bp.astype(ml_dtypes.bfloat16), float(r0), float(r3)


def _build_blur_sep(nc, N, C, H, W, r0, r3):
    OH, OW = H // 2, W // 2
    assert H == 256 and W == 256, "tuned for 256x256 spatial"
    groups_per_img = _group_sizes(N, C)

    x_ap = nc.dram_tensor("x", [N, C, H, W], F32, kind="ExternalInput").ap()
    b_ap = nc.dram_tensor("bmat", [128, 4, OH], BF16, kind="ExternalInput").ap()
    # transposed layout: host converts [n, oh, c, ow] -> [n, c, oh, ow]
    out_ap = nc.dram_tensor("out", [N, OH, C, OW], BF16, kind="ExternalOutput").ap()

    MUL = mybir.AluOpType.mult
    ADD = mybir.AluOpType.add
    have_r3 = abs(r3) > 1e-6
    have_r0 = abs(r0) > 1e-6

    with tile.TileContext(nc) as tc, ExitStack() as ctx:
        const_pool = ctx.enter_context(tc.tile_pool(name="const", bufs=1))
        x_pool = ctx.enter_context(tc.tile_pool(name="xt", bufs=5))
        acc_pool = ctx.enter_context(tc.tile_pool(name="acc", bufs=2))
        dr_pool = ctx.enter_context(tc.tile_pool(name="dr", bufs=2))
        t_pool = ctx.enter_context(tc.tile_pool(name="tb", bufs=2))
        # single PSUM tile per chunk, PH-MAJOR [oh, ph, c, k]: each (j, ph)
        # matmul writes exactly ONE full bank (4ch x 128 fp32 = 2KB), so
        # accumulation groups never alias another region's bank (a start=True
        # wipe is bank-scoped; the ch-major layout broke with 2-bank matmuls)
        psum_pool = ctx.enter_context(tc.tile_pool(name="po", bufs=2, space="PSUM"))

        # one small HWDGE load; no other on-device filter setup at all
        Bsb = const_pool.tile([128, 4, OH], BF16, tag="Bsb")
        nc.sync.dma_start(out=Bsb[:, :, :], in_=b_ap)

        gi = 0
        for n in range(N):
            c0 = 0
            for cg in groups_per_img[n]:
                # xt[p, c, r, w] holds x[n, c0+c, 2p+r, w]: 2 KB HBM runs
                xt = x_pool.tile(
                    [128, cg, 2, W], BF16, tag="xt", name=f"xt{gi}",
                    padded_shape=[128, 32, 2, W],
                )
                nc.gpsimd.dma_start(  # SWDGE: casts fp32 -> bf16
                    out=xt[:, :, :, :],
                    in_=x_ap[n, c0 : c0 + cg].rearrange("c (p r) w -> p c r w", r=2),
                )
                acc = acc_pool.tile(
                    [OH, cg, OW], BF16, tag="acc", name=f"acc{gi}",
                    padded_shape=[OH, 32, OW],
                )
                for pi, cp in enumerate(range(0, cg, 16)):  # 16-ch pair
                    sub = min(16, cg - cp)  # 8 for taper groups
                    # dr holds the drained planes of up to two 8-ch chunks:
                    # the post-PSUM ops then run once per PAIR (2x fewer DVE/
                    # Pool launches, halved per-op overhead; the tail of the
                    # run is DVE-paced so this directly shortens it)
                    dr = dr_pool.tile(
                        [OH, 2, sub, OW], BF16, tag="dr", name=f"dr{gi}_{cp}",
                        padded_shape=[OH, 2, 16, OW],
                    )
                    for ci in range(sub // 8):
                        cc = cp + 8 * ci
                        pt = psum_pool.tile(
                            [OH, 2, 8, OW], F32, tag="pt", name=f"pt{gi}_{cc}"
                        )
                        # (r, ph) combos palindromed across chunks so
                        # consecutive chunks share the boundary lhsT;
                        # 4-channel (1-full-bank) matmuls halve instructions
                        combos = [(0, 1), (1, 1), (0, 0), (1, 0)]
                        if (cc // 8) % 2 == 1:
                            combos = combos[::-1]
                        seen_ph = set()
                        for r, ph in combos:
                            lhsT = Bsb[:, 2 * r + ph, :]
                            for j in range(2):
                                ch = cc + 4 * j
                                nc.tensor.matmul(
                                    pt[:, ph, 4 * j : 4 * j + 4, :],
                                    lhsT=lhsT,
                                    rhs=xt[:, ch : ch + 4, r, ph : ph + 2 * OW - 1 : 2],
                                    start=(ph not in seen_ph),
                                    stop=(ph in seen_ph),
                                )
                            seen_ph.add(ph)
                        # drain this chunk's PSUM tile to its half of dr in
                        # ONE Scalar-engine op (psum frees right after)
                        nc.scalar.copy(
                            dr[:, :, 8 * ci : 8 * ci + 8, :], pt[:, :, :, :]
                        )
                    esb = dr[:, 0, :, :]
                    osb = dr[:, 1, :, :]
                    tt = t_pool.tile(
                        [OH, sub, OW], BF16, tag="tt", name=f"tt{gi}_{cp}",
                        padded_shape=[OH, 16, OW],
                    )
                    # tt = e' + o'  - plain add on the (otherwise idle) Pool
                    # engine; the scaled ops must stay on DVE (STT has no Pool
                    # support and no DVE fast mode)
                    nc.gpsimd.tensor_add(tt[:, :, :], esb, osb)
                    # tt[1:] += r3 * o'[:-1]
                    if have_r3:
                        nc.vector.scalar_tensor_tensor(
                            tt[:, :, 1:OW],
                            osb[:, :, 0 : OW - 1],
                            float(r3),
                            tt[:, :, 1:OW],
                            op0=MUL,
                            op1=ADD,
                        )
                    if have_r0:
                        # acc[:-1] = r0 * e'[1:] + tt[:-1]
                        nc.vector.scalar_tensor_tensor(
                            acc[:, cp : cp + sub, 0 : OW - 1],
                            esb[:, :, 1:OW],
                            float(r0),
                            tt[:, :, 0 : OW - 1],
                            op0=MUL,
                            op1=ADD,
                        )
                        # acc[OW-1] = tt[OW-1]  (no e' tap past the right edge)
                        nc.vector.tensor_copy(
                            acc[:, cp : cp + sub, OW - 1 : OW], tt[:, :, OW - 1 : OW]
                        )
                    else:
                        nc.vector.tensor_copy(acc[:, cp : cp + sub, :], tt[:, :, :])
                nc.sync.dma_start(
                    out=out_ap[n, :, c0 : c0 + cg, :], in_=acc[:, :, :]
                )
                c0 += cg
                gi += 1
    return nc


# ---------------------------------------------------------------------------
# Fallback path: direct 2D FIR (any f), B built on device - previous kernel
# ---------------------------------------------------------------------------


def _build_blur_direct(nc, N, C, H, W):
    OH, OW = H // 2, W // 2
    GROUPS_PER_IMG = [[32, 32, 32, 32]] * (N - 1) + [[32, 32, 32, 16, 8, 8]]
    assert all(sum(gs) == C for gs in GROUPS_PER_IMG)
    assert H == 256 and W == 256, "tuned for 256x256 spatial"

    x_ap = nc.dram_tensor("x", [N, C, H, W], F32, kind="ExternalInput").ap()
    f_ap = nc.dram_tensor("f", [4, 4], F32, kind="ExternalInput").ap()
    out_ap = nc.dram_tensor("out", [N, OH, C, OW], BF16, kind="ExternalOutput").ap()

    with tile.TileContext(nc) as tc, ExitStack() as ctx:
        const_pool = ctx.enter_context(tc.tile_pool(name="const", bufs=1))
        x_pool = ctx.enter_context(tc.tile_pool(name="xt", bufs=5))
        acc_pool = ctx.enter_context(tc.tile_pool(name="acc", bufs=3))
        psum_pool = ctx.enter_context(tc.tile_pool(name="po", bufs=8, space="PSUM"))

        f_sb = const_pool.tile([1, 16], F32, tag="f_sb")
        nc.sync.dma_start(out=f_sb[:, :], in_=f_ap.rearrange("a b -> (a b)"))
        f_bc = const_pool.tile([128, 16], F32, tag="f_bc")
        nc.gpsimd.partition_broadcast(f_bc[:, :], f_sb[:, :])

        ones = const_pool.tile([128, OH], F32, tag="ones")
        nc.gpsimd.memset(ones[:, :], 1.0)

        masks = {}
        for r in range(2):
            for dy in ((1, 3) if r == 0 else (0, 2)):
                m = const_pool.tile([128, OH], F32, tag=f"m{r}{dy}")
                nc.gpsimd.affine_select(
                    out=m[:, :],
                    in_=ones[:, :],
                    compare_op=mybir.AluOpType.is_equal,
                    fill=0.0,
                    base=r + 1 - dy,
                    channel_multiplier=2,
                    pattern=[[-2, OH]],
                )
                masks[(r, dy)] = m
        B = {}
        for r in range(2):
            dy_a, dy_b = (1, 3) if r == 0 else (0, 2)
            for dx in range(4):
                bf = const_pool.tile([128, OH], F32, tag=f"Bf{r}{dx}")
                fa = f_bc[:, 4 * (3 - dy_a) + (3 - dx) : 4 * (3 - dy_a) + (3 - dx) + 1]
                fb = f_bc[:, 4 * (3 - dy_b) + (3 - dx) : 4 * (3 - dy_b) + (3 - dx) + 1]
                nc.vector.tensor_scalar_mul(bf[:, :], masks[(r, dy_a)][:, :], fa)
                nc.vector.scalar_tensor_tensor(
                    bf[:, :],
                    masks[(r, dy_b)][:, :],
                    fb,
                    bf[:, :],
                    op0=mybir.AluOpType.mult,
                    op1=mybir.AluOpType.add,
                )
                br = const_pool.tile([128, OH], BF16, tag=f"B{r}{dx}")
                nc.gpsimd.dma_start(out=br[:, :], in_=bf[:, :])  # cast to bf16
                B[(r, dx)] = br

        DX_SLICE = {
            1: (0, OW, 0, OW),
            2: (1, OW, 0, OW),
            0: (1, OW - 1, 1, OW),
            3: (2, OW - 1, 0, OW - 1),
        }
        DX_ORDER = [1, 2, 0, 3]

        CG_MAX = max(max(gs) for gs in GROUPS_PER_IMG)
        gi = 0
        for n in range(N):
            c0 = 0
            for cg in GROUPS_PER_IMG[n]:
                xt = x_pool.tile(
                    [128, cg, 2, W], BF16, tag="xt", name=f"xt{gi}",
                    padded_shape=[128, CG_MAX, 2, W],
                )
                nc.gpsimd.dma_start(
                    out=xt[:, :, :, :],
                    in_=x_ap[n, c0 : c0 + cg].rearrange("c (p r) w -> p c r w", r=2),
                )
                acc = acc_pool.tile(
                    [OH, cg, OW], BF16, tag="acc", name=f"acc{gi}",
                    padded_shape=[OH, CG_MAX, OW],
                )
                for p0 in range(0, cg // 2, 8):
                    pch = min(8, cg // 2 - p0)
                    pos = [
                        psum_pool.tile([OH, 2, OW], F32, tag="po", name=f"po{t}")
                        for t in range(pch)
                    ]
                    for ri in range(2):
                        for di, dx in enumerate(DX_ORDER):
                            ws, wl, o0, o1 = DX_SLICE[dx]
                            lhsT = B[(ri, dx)]
                            for jj in range(pch):
                                j = p0 + jj
                                nc.tensor.matmul(
                                    pos[jj][:, :, o0:o1],
                                    lhsT=lhsT[:, :],
                                    rhs=xt[
                                        :, 2 * j : 2 * j + 2, ri,
                                        ws : ws + 2 * wl - 1 : 2,
                                    ],
                                    start=(ri == 0 and di == 0),
                                    stop=(ri == 1 and di == 3),
                                )
                    for t in range(pch):
                        dst = acc[:, 2 * (p0 + t) : 2 * (p0 + t) + 2, :]
                        if t % 2 == 0:
                            nc.vector.tensor_copy(dst, pos[t][:, :, :])
                        else:
                            nc.scalar.copy(dst, pos[t][:, :, :])
                nc.sync.dma_start(
                    out=out_ap[n, :, c0 : c0 + cg, :], in_=acc[:, :, :]
                )
                c0 += cg
                gi += 1
    return nc


# ---------------------------------------------------------------------------


_PROGRAM_CACHE = {}


def _get_program(key, builder):
    if key not in _PROGRAM_CACHE:
        nc = bacc.Bacc(
            "TRN2", target_bir_lowering=False, debug=False, num_devices=N_CORES
        )
        builder(nc)
        nc.compile()
        _PROGRAM_CACHE[key] = nc
    return _PROGRAM_CACHE[key]


def _run(x, f, trace=False, tmpdir=None):
    x = np.ascontiguousarray(x, dtype=np.float32)
    f = np.ascontiguousarray(f, dtype=np.float32)
    N, C, H, W = x.shape
    assert N % N_CORES == 0, f"batch {N} not divisible by {N_CORES} cores"
    nb = N // N_CORES

    sep = _host_separable(f)
    if sep is not None:
        bmat, r0, r3 = sep
        key = ("sep", x.shape, round(r0, 9), round(r3, 9))
        nc = _get_program(
            key, lambda nc: _build_blur_sep(nc, nb, C, H, W, r0, r3)
        )
        in_maps = [
            {"x": x[k * nb : (k + 1) * nb], "bmat": bmat} for k in range(N_CORES)
        ]
    else:
        key = ("direct", x.shape)
        nc = _get_program(key, lambda nc: _build_blur_direct(nc, nb, C, H, W))
        in_maps = [
            {"x": x[k * nb : (k + 1) * nb], "f": f} for k in range(N_CORES)
        ]

    res = run_bass_kernel_spmd(
        nc, in_maps, core_ids=list(range(N_CORES)), trace=trace, tmpdir=tmpdir
    )
    # results are [nb, OH, C, OW] bf16; reassemble to [N, C, OH, OW] fp32
    out_t = np.concatenate(
        [np.asarray(res.results[k]["out"]) for k in range(N_CORES)], axis=0
    )
    out = out_t.transpose(0, 2, 1, 3).astype(np.float32)
    return np.ascontiguousarray(out), res


def kernel(x, f):
    out, _ = _run(x, f)
    return out


# revision 20
# speedup vs baseline: 1.0672x; 1.0672x over previous
"""BlurDownsample (depthwise 4x4 FIR + 2x downsample) on 8 TRN2 NeuronCores.

Contract: kernel(x, f) takes the FULL inputs
    x: [16, 128, 256, 256] float32,  f: [4, 4] float32
and returns the FULL output [16, 128, 128, 128] float32, matching
    upfirdn2d(x, f, down=2, padding=(1, 1), flip_filter=False):
    out[n,c,oy,ox] = sum_{dy,dx in 0..3} f[3-dy, 3-dx] * x[2oy+dy-1, 2ox+dx-1]
(out-of-range x indices read as zero).

Sharding: pure data-parallel over the batch - core k processes
x[2k:2k+2]; outputs are reassembled on the host.

Fast path (separable filter, used for the graded f = outer([1,3,3,1])^2):
  f is SVD-decomposed on the HOST into rank-1 f = fv ⊗ fh. The device
  program then does:
  - Vertical FIR on the Tensor engine: banded bf16 matrices
        B[r,ph][p, oh] = fv[3-dy] * s_ph,   dy = 2p + r - 2oh + 1
    (s_ph folds the horizontal center taps fh[2]/fh[1] in) contract the
    input-row pairs held on partitions. rhs is the even/odd (ph) column
    phase of x, so PSUM receives two planes per channel:
        e'[oh,k] = fh[2] * vert(x)[oh, 2k],  o'[oh,k] = fh[1] * vert[2k+1]
    Only 4 matmuls x 256 columns per channel pair (vs 8 in the direct
    2D formulation) - each input column is streamed exactly once, which
    halves Tensor-engine columns/power. That matters because the
    baseline trace showed HAM power throttling (k=4/8 duty for ~57us)
    gating the Tensor engine and stalling the whole load pipeline.
  - PSUM layout is PH-MAJOR [oh, ph, c, k] so each 4-channel matmul fills
    exactly ONE 2KB PSUM bank (a start=True reset is bank-scoped; mixed-
    region banks with multi-bank matmuls corrupt accumulation).
  - Horizontal 4-tap combine, engine-balanced (measured: DVE STT has no
    fast mode ~1.2ns/elem, DVE TT gets 2x for bf16 SBUF, Pool only does
    plain TT at 0.42 efficiency, only DVE/Act can read PSUM):
      Scalar engine: ONE activation copy drains the whole PSUM tile to
        bf16 SBUF (psum frees immediately -> short cross-engine chain);
      Pool (gpsimd): tt = e' + o' (plain add);
      DVE: tt[1:] += (fh[3]/fh[1]) o'[:-1],
           acc[:-1] = (fh[0]/fh[2]) e'[1:] + tt[:-1], 1-col fixup.
    This caps every engine at ~83us/core: the chip HAM power throttle
    (k=4/8 duty windows, 40-60% of runtime on a busy box) gates compute
    engines but NOT the DMA queues, so exec ~ head + maxEngine/(1-thr)
    + tail; balance matters more than total work.
  - B matrices are computed on the host and passed as a bf16 input:
    ZERO on-device filter setup, so the first matmul only waits for the
    first (deliberately small) x group - the baseline lost 54us waiting
    for B cast-DMAs queued behind 8 MiB loads.
  - x is cast fp32 -> bf16 inside the load DMA (SWDGE on the gpsimd
    queue) with two adjacent H rows per partition => 2 KB contiguous
    HBM bursts (~410 GB/s observed). Group sizes taper 8,8,16,32,... at
    the start (compute starts ~11us in) and ...,16,8,8 at the end
    (short drain tail).
  - Output is stored bf16 in [n, oh, c, ow] layout (4-8 KB contiguous
    runs on the sync HWDGE queue); the host transposes back and
    upcasts (rel err ~4e-3 vs the 2e-2 gate).

Fallback path (non-separable f): the previous direct-2D kernel (whole
4x4 FIR as 8 banded matmuls per channel pair, B built on device).

Roofline: 72 MiB HBM traffic (64 read + 8 write) at the ~410-435 GB/s
per-core DMA fabric rate = 174-184us, plus ~10us head/tail.
"""

from contextlib import ExitStack

import numpy as np
import ml_dtypes

import concourse.tile as tile
from concourse import bacc, mybir
from concourse.bass_utils import run_bass_kernel_spmd

F32 = mybir.dt.float32
BF16 = mybir.dt.bfloat16

N_CORES = 8


def _group_sizes(nb, C):
    """Per-image channel-group sizes: taper up at the very start (so the
    first matmul starts early) and down at the very end (short tail)."""
    assert C == 128, "tuned for C=128"
    head = [8, 8, 16, 32, 32, 32]
    full = [32, 32, 32, 32]
    tail = [32, 32, 32, 16, 8, 8]
    both = [8, 8, 16, 32, 32, 16, 8, 8]
    if nb == 1:
        imgs = [both]
    else:
        imgs = [head] + [full] * (nb - 2) + [tail]
    assert all(sum(g) == C for g in imgs)
    return imgs


# ---------------------------------------------------------------------------
# Fast path: separable filter, host-built B matrices
# ---------------------------------------------------------------------------


def _host_separable(f):
    """Rank-1 decomposition of f with folded horizontal scales.

    Returns (B_host [128, 4, 128] bf16, r0, r3) or None if f is not
    (numerically) rank-1 / has degenerate center taps.
    """
    f64 = np.asarray(f, dtype=np.float64)
    U, S, Vt = np.linalg.svd(f64)
    if S[0] == 0.0 or S[1] > 1e-5 * S[0]:
        return None
    fv = U[:, 0] * S[0]
    fh = Vt[0, :]
    if fh.sum() < 0:
        fv, fh = -fv, -fh
    m = np.abs(fh).max()
    se, so = fh[2], fh[1]
    if abs(se) < 1e-3 * m or abs(so) < 1e-3 * m:
        return None
    r0 = fh[0] / se
    r3 = fh[3] / so
    # B[r,ph][p,oh] = fv[3-dy] * s_ph, dy = 2p + r - 2oh + 1 (0<=dy<=3)
    p = np.arange(128)[:, None]
    oh = np.arange(128)[None, :]
    B = np.zeros((2, 2, 128, 128), np.float64)
    for r in range(2):
        dy = 2 * p + r - 2 * oh + 1
        valid = (dy >= 0) & (dy <= 3)
        vals = np.where(valid, fv[np.clip(3 - dy, 0, 3)], 0.0)
        B[r, 0] = vals * se
        B[r, 1] = vals * so
    # pack to [p, b=2r+ph, oh]
    Bp = np.ascontiguousarray(np.transpose(B, (2, 0, 1, 3)).reshape(128, 4, 128))
    return Bp.astype(ml_dtypes.bfloat16), float(r0), float(r3)


def _build_blur_sep(nc, N, C, H, W, r0, r3):
    OH, OW = H // 2, W // 2
    assert H == 256 and W == 256, "tuned for 256x256 spatial"
    groups_per_img = _group_sizes(N, C)

    x_ap = nc.dram_tensor("x", [N, C, H, W], F32, kind="ExternalInput").ap()
    b_ap = nc.dram_tensor("bmat", [128, 4, OH], BF16, kind="ExternalInput").ap()
    # transposed layout: host converts [n, oh, c, ow] -> [n, c, oh, ow]
    out_ap = nc.dram_tensor("out", [N, OH, C, OW], BF16, kind="ExternalOutput").ap()

    MUL = mybir.AluOpType.mult
    ADD = mybir.AluOpType.add
    have_r3 = abs(r3) > 1e-6
    have_r0 = abs(r0) > 1e-6

    with tile.TileContext(nc) as tc, ExitStack() as ctx:
        const_pool = ctx.enter_context(tc.tile_pool(name="const", bufs=1))
        x_pool = ctx.enter_context(tc.tile_pool(name="xt", bufs=5))
        acc_pool = ctx.enter_context(tc.tile_pool(name="acc", bufs=3))
        dr_pool = ctx.enter_context(tc.tile_pool(name="dr", bufs=3))
        t_pool = ctx.enter_context(tc.tile_pool(name="tb", bufs=2))
        # single PSUM tile per chunk, PH-MAJOR [oh, ph, c, k]: each (j, ph)
        # matmul writes exactly ONE full bank (4ch x 128 fp32 = 2KB), so
        # accumulation groups never alias another region's bank (a start=True
        # wipe is bank-scoped; the ch-major layout broke with 2-bank matmuls)
        psum_pool = ctx.enter_context(tc.tile_pool(name="po", bufs=2, space="PSUM"))

        # one small HWDGE load; no other on-device filter setup at all
        Bsb = const_pool.tile([128, 4, OH], BF16, tag="Bsb")
        nc.sync.dma_start(out=Bsb[:, :, :], in_=b_ap)

        gi = 0
        for n in range(N):
            c0 = 0
            for cg in groups_per_img[n]:
                # xt[p, c, r, w] holds x[n, c0+c, 2p+r, w]: 2 KB HBM runs
                xt = x_pool.tile(
                    [128, cg, 2, W], BF16, tag="xt", name=f"xt{gi}",
                    padded_shape=[128, 32, 2, W],
                )
                nc.gpsimd.dma_start(  # SWDGE: casts fp32 -> bf16
                    out=xt[:, :, :, :],
                    in_=x_ap[n, c0 : c0 + cg].rearrange("c (p r) w -> p c r w", r=2),
                )
                acc = acc_pool.tile(
                    [OH, cg, OW], BF16, tag="acc", name=f"acc{gi}",
                    padded_shape=[OH, 32, OW],
                )
                for ci, cc in enumerate(range(0, cg, 8)):  # 8-ch chunk = 4 banks
                    pt = psum_pool.tile(
                        [OH, 2, 8, OW], F32, tag="pt", name=f"pt{gi}_{cc}"
                    )
                    # (r, ph) combos palindromed across chunks so consecutive
                    # chunks share the boundary lhsT (saves a LDWEIGHTS);
                    # 4-channel (1-full-bank) matmuls halve instruction count
                    combos = [(0, 1), (1, 1), (0, 0), (1, 0)]
                    if ci % 2 == 1:
                        combos = combos[::-1]
                    seen_ph = set()
                    for r, ph in combos:
                        lhsT = Bsb[:, 2 * r + ph, :]
                        for j in range(2):
                            ch = cc + 4 * j
                            nc.tensor.matmul(
                                pt[:, ph, 4 * j : 4 * j + 4, :],
                                lhsT=lhsT,
                                rhs=xt[:, ch : ch + 4, r, ph : ph + 2 * OW - 1 : 2],
                                start=(ph not in seen_ph),
                                stop=(ph in seen_ph),
                            )
                        seen_ph.add(ph)
                    # drain the whole PSUM tile to bf16 SBUF in ONE Scalar-
                    # engine op (psum frees right after; DVE then runs all-
                    # SBUF bf16 ops in its 2x fast mode)
                    dr = dr_pool.tile(
                        [OH, 2, 8, OW], BF16, tag="dr", name=f"dr{gi}_{cc}"
                    )
                    nc.scalar.copy(dr[:, :, :, :], pt[:, :, :, :])
                    esb = dr[:, 0, :, :]
                    osb = dr[:, 1, :, :]
                    tt = t_pool.tile([OH, 8, OW], BF16, tag="tt", name=f"tt{gi}_{cc}")
                    # tt = e' + o'  - plain add on the (otherwise idle) Pool
                    # engine; the scaled ops must stay on DVE (STT has no Pool
                    # support and no DVE fast mode; plain TT is 2x on DVE but
                    # DVE is the busier engine)
                    nc.gpsimd.tensor_add(tt[:, :, :], esb, osb)
                    # tt[1:] += r3 * o'[:-1]
                    if have_r3:
                        nc.vector.scalar_tensor_tensor(
                            tt[:, :, 1:OW],
                            osb[:, :, 0 : OW - 1],
                            float(r3),
                            tt[:, :, 1:OW],
                            op0=MUL,
                            op1=ADD,
                        )
                    if have_r0:
                        # acc[:-1] = r0 * e'[1:] + tt[:-1]
                        nc.vector.scalar_tensor_tensor(
                            acc[:, cc : cc + 8, 0 : OW - 1],
                            esb[:, :, 1:OW],
                            float(r0),
                            tt[:, :, 0 : OW - 1],
                            op0=MUL,
                            op1=ADD,
                        )
                        # acc[OW-1] = tt[OW-1]  (no e' tap past the right edge)
                        nc.vector.tensor_copy(
                            acc[:, cc : cc + 8, OW - 1 : OW], tt[:, :, OW - 1 : OW]
                        )
                    else:
                        nc.vector.tensor_copy(acc[:, cc : cc + 8, :], tt[:, :, :])
                nc.sync.dma_start(
                    out=out_ap[n, :, c0 : c0 + cg, :], in_=acc[:, :, :]
                )
                c0 += cg
                gi += 1
    return nc


# ---------------------------------------------------------------------------
# Fallback path: direct 2D FIR (any f), B built on device - previous kernel
# ---------------------------------------------------------------------------


def _build_blur_direct(nc, N, C, H, W):
    OH, OW = H // 2, W // 2
    GROUPS_PER_IMG = [[32, 32, 32, 32]] * (N - 1) + [[32, 32, 32, 16, 8, 8]]
    assert all(sum(gs) == C for gs in GROUPS_PER_IMG)
    assert H == 256 and W == 256, "tuned for 256x256 spatial"

    x_ap = nc.dram_tensor("x", [N, C, H, W], F32, kind="ExternalInput").ap()
    f_ap = nc.dram_tensor("f", [4, 4], F32, kind="ExternalInput").ap()
    out_ap = nc.dram_tensor("out", [N, OH, C, OW], BF16, kind="ExternalOutput").ap()

    with tile.TileContext(nc) as tc, ExitStack() as ctx:
        const_pool = ctx.enter_context(tc.tile_pool(name="const", bufs=1))
        x_pool = ctx.enter_context(tc.tile_pool(name="xt", bufs=5))
        acc_pool = ctx.enter_context(tc.tile_pool(name="acc", bufs=3))
        psum_pool = ctx.enter_context(tc.tile_pool(name="po", bufs=8, space="PSUM"))

        f_sb = const_pool.tile([1, 16], F32, tag="f_sb")
        nc.sync.dma_start(out=f_sb[:, :], in_=f_ap.rearrange("a b -> (a b)"))
        f_bc = const_pool.tile([128, 16], F32, tag="f_bc")
        nc.gpsimd.partition_broadcast(f_bc[:, :], f_sb[:, :])

        ones = const_pool.tile([128, OH], F32, tag="ones")
        nc.gpsimd.memset(ones[:, :], 1.0)

        masks = {}
        for r in range(2):
            for dy in ((1, 3) if r == 0 else (0, 2)):
                m = const_pool.tile([128, OH], F32, tag=f"m{r}{dy}")
                nc.gpsimd.affine_select(
                    out=m[:, :],
                    in_=ones[:, :],
                    compare_op=mybir.AluOpType.is_equal,
                    fill=0.0,
                    base=r + 1 - dy,
                    channel_multiplier=2,
                    pattern=[[-2, OH]],
                )
                masks[(r, dy)] = m
        B = {}
        for r in range(2):
            dy_a, dy_b = (1, 3) if r == 0 else (0, 2)
            for dx in range(4):
                bf = const_pool.tile([128, OH], F32, tag=f"Bf{r}{dx}")
                fa = f_bc[:, 4 * (3 - dy_a) + (3 - dx) : 4 * (3 - dy_a) + (3 - dx) + 1]
                fb = f_bc[:, 4 * (3 - dy_b) + (3 - dx) : 4 * (3 - dy_b) + (3 - dx) + 1]
                nc.vector.tensor_scalar_mul(bf[:, :], masks[(r, dy_a)][:, :], fa)
                nc.vector.scalar_tensor_tensor(
                    bf[:, :],
                    masks[(r, dy_b)][:, :],
                    fb,
                    bf[:, :],
                    op0=mybir.AluOpType.mult,
                    op1=mybir.AluOpType.add,
                )
                br = const_pool.tile([128, OH], BF16, tag=f"B{r}{dx}")
                nc.gpsimd.dma_start(out=br[:, :], in_=bf[:, :])  # cast to bf16
                B[(r, dx)] = br

        DX_SLICE = {
            1: (0, OW, 0, OW),
            2: (1, OW, 0, OW),
            0: (1, OW - 1, 1, OW),
            3: (2, OW - 1, 0, OW - 1),
        }
        DX_ORDER = [1, 2, 0, 3]

        CG_MAX = max(max(gs) for gs in GROUPS_PER_IMG)
        gi = 0
        for n in range(N):
            c0 = 0
            for cg in GROUPS_PER_IMG[n]:
                xt = x_pool.tile(
                    [128, cg, 2, W], BF16, tag="xt", name=f"xt{gi}",
                    padded_shape=[128, CG_MAX, 2, W],
                )
                nc.gpsimd.dma_start(
                    out=xt[:, :, :, :],
                    in_=x_ap[n, c0 : c0 + cg].rearrange("c (p r) w -> p c r w", r=2),
                )
                acc = acc_pool.tile(
                    [OH, cg, OW], BF16, tag="acc", name=f"acc{gi}",
                    padded_shape=[OH, CG_MAX, OW],
                )
                for p0 in range(0, cg // 2, 8):
                    pch = min(8, cg // 2 - p0)
                    pos = [
                        psum_pool.tile([OH, 2, OW], F32, tag="po", name=f"po{t}")
                        for t in range(pch)
                    ]
                    for ri in range(2):
                        for di, dx in enumerate(DX_ORDER):
                            ws, wl, o0, o1 = DX_SLICE[dx]
                            lhsT = B[(ri, dx)]
                            for jj in range(pch):
                                j = p0 + jj
                                nc.tensor.matmul(
                                    pos[jj][:, :, o0:o1],
                                    lhsT=lhsT[:, :],
                                    rhs=xt[
                                        :, 2 * j : 2 * j + 2, ri,
                                        ws : ws + 2 * wl - 1 : 2,
                                    ],
                                    start=(ri == 0 and di == 0),
                                    stop=(ri == 1 and di == 3),
                                )
                    for t in range(pch):
                        dst = acc[:, 2 * (p0 + t) : 2 * (p0 + t) + 2, :]
                        if t % 2 == 0:
                            nc.vector.tensor_copy(dst, pos[t][:, :, :])
                        else:
                            nc.scalar.copy(dst, pos[t][:, :, :])
                nc.sync.dma_start(
                    out=out_ap[n, :, c0 : c0 + cg, :], in_=acc[:, :, :]
                )
                c0 += cg
                gi += 1
    return nc


# ---------------------------------------------------------------------------


_PROGRAM_CACHE = {}


def _get_program(key, builder):
    if key not in _PROGRAM_CACHE:
        nc = bacc.Bacc(
            "TRN2", target_bir_lowering=False, debug=False, num_devices=N_CORES
        )
        builder(nc)
        nc.compile()
        _PROGRAM_CACHE[key] = nc
    return _PROGRAM_CACHE[key]


def _run(x, f, trace=False, tmpdir=None):
    x = np.ascontiguousarray(x, dtype=np.float32)
    f = np.ascontiguousarray(f, dtype=np.float32)
    N, C, H, W = x.shape
    assert N % N_CORES == 0, f"batch {N} not divisible by {N_CORES} cores"
    nb = N // N_CORES

    sep = _host_separable(f)
    if sep is not None:
        bmat, r0, r3 = sep
        key = ("sep", x.shape, round(r0, 9), round(r3, 9))
        nc = _get_program(
            key, lambda nc: _build_blur_sep(nc, nb, C, H, W, r0, r3)
        )
        in_maps = [
            {"x": x[k * nb : (k + 1) * nb], "bmat": bmat} for k in range(N_CORES)
        ]
    else:
        key = ("direct", x.shape)
        nc = _get_program(key, lambda nc: _build_blur_direct(nc, nb, C, H, W))
        in_maps = [
            {"x": x[k * nb : (k + 1) * nb], "f": f} for k in range(N_CORES)
        ]

    res = run_bass_kernel_spmd(
        nc, in_maps, core_ids=list(range(N_CORES)), trace=trace, tmpdir=tmpdir
    )
    # results are [nb, OH, C, OW] bf16; reassemble to [N, C, OH, OW] fp32
    out_t = np.concatenate(
        [np.asarray(res.results[k]["out"]) for k in range(N_CORES)], axis=0
    )
    out = out_t.transpose(0, 2, 1, 3).astype(np.float32)
    return np.ascontiguousarray(out), res


def kernel(x, f):
    out, _ = _run(x, f)
    return out
